# revision 65
# baseline (speedup 1.0000x reference)
"""GCN encoder (2x GCNConv + global_mean_pool + two linear heads) on 8 trn2 cores.

Strategy (SPMD, one program, per-core data):
  - 1024 graphs -> 128 graphs/core; nodes re-indexed into a padded per-core
    layout (SLICE = NT*128 rows/core, PN = 8*SLICE).
  - Tables stored as single bf16 [rows, 128] (256B rows, min dma_gather
    granularity).  Message passing gathers h~[src] rows with dma_gather
    (dst-sorted edge chunks of 128, int16 indices relative to a <=32K-row
    window), builds one-hot routing matrices (batched is_equal vs iota) and
    accumulates onehot.T @ gathered into PSUM per (window, dst-tile) group,
    merged into SBUF f32 accumulators.
  - conv1 uses a per-core ROTATED layout (own slice first): Phase A computes
    x@W1 (bf16) for the full table redundantly per core; local tiles also
    init the SBUF accumulators (covers the self-loop term, no indirect DMA).
    tab1 is split into NW window tensors so gathers overlap Phase A's tail.
  - conv2: per dst-tile after conv1 mp: relu -> transpose -> @W2 -> local
    h2~ tiles (acc init for conv2) + bf16 copies AllGathered chunk-by-chunk
    (NCH=4 chunked collectives, overlapped with conv1 mp of later blocks).
    conv2 mp is window-major (chunk-tensor-major) so windows 0..2 hide the
    tail AllGather; conv1 mp is block-major (TCHP tiles/block, per-tile PSUM
    chain across windows; each open chain owns a full PSUM bank because
    start=True clears has_written bank-wide).
  - Most messages ride IDENTITY chunks (slot==dst-local, constant identity
    lhsT, no one-hot build); only per-dst overflow edges get DVE-built
    one-hots.  Identity padding gathers a dedicated zero row.
  - Pooling: one-hot by graph-local id, matmul accumulate; per-graph counts
    are host-precomputed.  Outputs per core: mu/logvar for its 128 graphs.
"""

import numpy as np
import ml_dtypes

import concourse.bass as bass
import concourse.bacc as bacc
import concourse.mybir as mybir
import concourse.tile as tile
from concourse.bass_utils import run_bass_kernel_spmd

BF16 = ml_dtypes.bfloat16
NCORES = 8
NW = 4  # conv1 gather windows (window = 2*SLICE rows, fits int16)
NCH = 4  # conv2 chunk tensors == AllGather chunks == processing blocks
PAD_DL = 200.0  # one-hot miss marker (exact in bf16, outside 0..127)
GC = 64  # max chunks (of 128 gathered rows) per dma_gather call
LID = 5  # identity chunks per (window, tile) group
TCHP = 4  # tiles per processing block (each open PSUM chain owns a bank:
          # start=True clears has_written for the WHOLE bank, so interleaved
          # chains must never share one)


def _cdiv(a, b):
    return -(-a // b)


def _build_stream(owner, w, srel, lt, dloc, NWIN, NT, TCH, L, zrows):
    """Edge stream in (block, window, tile) order with shared chunk quotas.

    Per (window, tile) group: the first min(L, .) messages of every dst node
    go to IDENTITY chunks (slot == dst-local, no one-hot needed; short nodes
    pad with the window's zero row); overflow edges pack densely into one-hot
    chunks.  Returns Qid/Qoh [NWIN, NT], offs, calls, CH, dl, idx16."""
    E = owner.shape[0]
    # per-dst counts within each (owner, window, tile) group
    cnt4 = np.zeros((NCORES, NWIN, NT, 128), np.int32)
    np.add.at(cnt4, (owner, w, lt, dloc), 1)
    maxn = cnt4.max(axis=3).max(axis=0)  # [NWIN, NT]
    # pick identity level per window: minimize gather chunks + ~0.7x the
    # one-hot chunks (their extra DVE build cost relative to a chunk's DMA)
    Lw = np.zeros(NWIN, np.int64)
    for w_ in range(NWIN):
        best = None
        for Lc in range(0, L + 3):
            qid = np.minimum(Lc, maxn[w_])
            over = np.maximum(cnt4[:, w_] - Lc, 0).sum(axis=2)
            qoh = _cdiv(over.max(axis=0), 128)
            cost = (qid + qoh).sum() + 0.73 * qoh.sum()
            if best is None or cost < best[0]:
                best = (cost, Lc)
        Lw[w_] = best[1]
    Qid = np.minimum(Lw[:, None], maxn).astype(np.int64)
    Lpere = Lw[w]  # per-edge identity level
    nover = np.maximum(cnt4 - Lw[None, :, None, None], 0).sum(axis=3)
    Qoh = _cdiv(nover.max(axis=0), 128).astype(np.int64)
    Q = Qid + Qoh

    NBLK = _cdiv(NT, TCH)
    offs = np.zeros((NWIN, NT), np.int64)
    calls = []
    chunk_w = []
    c = 0
    for b in range(NBLK):
        t0, t1 = b * TCH, min((b + 1) * TCH, NT)
        for w_ in range(NWIN):
            span0 = c
            for t in range(t0, t1):
                offs[w_, t] = c
                c += int(Q[w_, t])
                chunk_w.extend([w_] * int(Q[w_, t]))
            n = c - span0
            ncalls = _cdiv(n, GC)
            cc = span0
            for i in range(ncalls):
                sz = n // ncalls + (1 if i < n % ncalls else 0)
                calls.append((b, w_, cc, cc + sz))
                cc += sz
    CH = c

    blk = lt // TCH
    order = np.lexsort((dloc, lt, w, blk, owner))
    so, sw, st = owner[order], w[order], lt[order]
    sr = srel[order].astype(np.int64)
    sd = dloc[order]
    # rank within (owner, w, t, dst)
    grpd = ((so * NWIN + sw) * NT + st) * 128 + sd
    start_of = np.zeros(E, np.int64)
    is_new = np.ones(E, bool)
    is_new[1:] = grpd[1:] != grpd[:-1]
    start_of[is_new] = np.arange(E)[is_new]
    start_of = np.maximum.accumulate(start_of)
    r = np.arange(E) - start_of

    pos = np.empty(E, np.int64)
    sel1 = r < Lpere[order]
    pos[sel1] = (offs[sw[sel1], st[sel1]] + r[sel1]) * 128 + sd[sel1]
    # overflow edges: dense rank within (owner, w, t)
    i2 = np.flatnonzero(~sel1)
    grp2 = (so[i2] * NWIN + sw[i2]) * NT + st[i2]
    st2 = np.zeros(len(i2), np.int64)
    isn2 = np.ones(len(i2), bool)
    isn2[1:] = grp2[1:] != grp2[:-1]
    st2[isn2] = np.arange(len(i2))[isn2]
    st2 = np.maximum.accumulate(st2)
    r2 = np.arange(len(i2)) - st2
    pos[i2] = (offs[sw[i2], st[i2]] + Qid[sw[i2], st[i2]]) * 128 + r2

    dl = np.full((NCORES, 128, CH), PAD_DL, np.float32)
    dl[so[i2], pos[i2] % 128, pos[i2] // 128] = sd[i2]
    # default index = the window's zero row (identity-chunk padding must
    # contribute zero); one-hot padding also lands there harmlessly
    zc = np.asarray(zrows, np.int16)[np.asarray(chunk_w, np.int64)]  # [CH]
    V = np.tile(np.repeat(zc, 128)[None, :], (NCORES, 1))
    V[so, pos] = sr
    idx16 = np.zeros((NCORES, 128, CH * 8), np.int16)
    for cc in range(NCORES):
        for (_, _, c0, c1) in calls:
            b_ = V[cc, c0 * 128 : c1 * 128].reshape(-1, 16).T  # [16, ncols]
            for g in range(8):
                idx16[cc, g * 16 : (g + 1) * 16, c0 * 8 : c1 * 8] = b_
    return dict(Qid=Qid, Qoh=Qoh, Q=Q, offs=offs, calls=calls, CH=CH,
                dl=dl.astype(BF16), idx16=idx16)


def prep_host(x, edge_index, batch, n_graphs):
    x = np.asarray(x, np.float32)
    edge_index = np.asarray(edge_index)
    batch = np.asarray(batch).astype(np.int64)
    N, F = x.shape
    gpc = n_graphs // NCORES

    core_of_node = (batch // gpc).astype(np.int64)  # sorted non-decreasing
    counts = np.bincount(core_of_node, minlength=NCORES)
    starts = np.zeros(NCORES + 1, np.int64)
    starts[1:] = np.cumsum(counts)
    NT = int(_cdiv(int(counts.max()), 128))
    SLICE = NT * 128
    assert 2 * SLICE <= 32767, "window exceeds int16 range"
    PN = NCORES * SLICE
    NTOT = PN // 128
    TCH = _cdiv(NT, NCH)
    tch = [min(TCH, NT - k * TCH) for k in range(NCH)]
    assert all(t > 0 for t in tch), f"bad chunking {tch}"
    assert max(tch) * 128 * NCORES + 128 <= 32767, "chunk tensor exceeds int16"
    cstart = np.zeros(NCH + 1, np.int64)
    cstart[1:] = np.cumsum(tch)

    rank = np.empty(N, np.int64)
    for c in range(NCORES):
        rank[starts[c] : starts[c + 1]] = np.arange(counts[c])
    tile_of = rank // 128
    part_of = rank % 128

    src = edge_index[0].astype(np.int64)
    dst = edge_index[1].astype(np.int64)
    deg = np.bincount(dst, minlength=N).astype(np.float32) + 1.0  # + self loop

    owner = core_of_node[dst]
    lt = tile_of[dst]
    dloc = part_of[dst]

    # conv1: rotated layout (own slice first); window = 2 consecutive slices
    rel = (core_of_node[src] - owner) % NCORES
    w1 = rel >> 1
    srel1 = (rel & 1) * SLICE + rank[src]
    s1 = _build_stream(owner, w1, srel1, lt, dloc, NW, NT, TCHP,
                       LID, [2 * SLICE] * NW)

    # conv2: standard layout, chunk tensors over tile ranges
    k2 = np.searchsorted(cstart, tile_of[src], side="right") - 1
    tch_arr = np.asarray(tch, np.int64)
    srel2 = (core_of_node[src] * (tch_arr[k2] * 128)
             + (rank[src] - cstart[k2] * 128))
    # conv2 stream is window-major (single block): windows 0..2 are ready
    # early, so conv2 work hides the tail AllGather
    s2 = _build_stream(owner, k2, srel2, lt, dloc, NCH, NT, NT,
                       LID, [NCORES * t * 128 for t in tch])

    # standard padded transposed features + degree table, then per-core rotate
    xT_std = np.zeros((F, PN), BF16)
    pid = core_of_node * SLICE + rank
    xT_std[:, pid] = x.T.astype(BF16)
    degw_std = np.ones((128, NTOT), np.float32)
    degw_std[part_of, pid // 128] = deg
    xTs = np.stack([
        np.concatenate([xT_std[:, c * SLICE :], xT_std[:, : c * SLICE]], axis=1)
        for c in range(NCORES)
    ])
    degws = np.stack([
        np.concatenate([degw_std[:, c * NT :], degw_std[:, : c * NT]], axis=1)
        for c in range(NCORES)
    ])

    glocw = np.full((NCORES, 128, NT), PAD_DL, np.float32)
    glocw[core_of_node, part_of, tile_of] = (batch % gpc).astype(np.float32)

    # per-graph node counts (for the pooling mean) are host-derivable
    gcnt = np.bincount(batch, minlength=n_graphs).astype(np.float32)
    icnt = (1.0 / np.maximum(gcnt, 1.0)).reshape(NCORES, 128, 1)

    return dict(NT=NT, PN=PN, F=F, tch=tch, cstart=cstart, s1=s1, s2=s2,
                xTs=xTs, degws=degws, glocw=glocw, icnt=icnt)


def build_nc(pr, LAT, stop_after=None):
    dt = mybir.dt
    f32, bf16, i16 = dt.float32, dt.bfloat16, dt.int16
    NT, PN, F = pr["NT"], pr["PN"], pr["F"]
    tch, cstart = pr["tch"], pr["cstart"]
    NTOT = PN // 128
    NB = PN // 512
    WT = (PN // NW) // 128  # tiles per conv1 window tensor
    s1, s2 = pr["s1"], pr["s2"]
    CH1, CH2 = s1["CH"], s2["CH"]
    QMAX = int(max(s1["Qoh"].max(), s2["Qoh"].max(), 1))
    NBLKP = _cdiv(NT, TCHP)

    def _call_map(calls):
        m = {}
        for (_, _, c0, c1) in calls:
            for q in range(c0, c1):
                m[q] = (c0, q - c0)
        return m

    cmap1, cmap2 = _call_map(s1["calls"]), _call_map(s2["calls"])
    AF = mybir.ActivationFunctionType
    OP = mybir.AluOpType

    nc = bacc.Bacc(num_swdge_queues=2)
    xT_d = nc.declare_dram_parameter("xT", [F, PN], bf16, False)
    idx1_d = nc.declare_dram_parameter("idx1", [128, CH1 * 8], i16, False)
    dl1_d = nc.declare_dram_parameter("dl1", [128, CH1], bf16, False)
    idx2_d = nc.declare_dram_parameter("idx2", [128, CH2 * 8], i16, False)
    dl2_d = nc.declare_dram_parameter("dl2", [128, CH2], bf16, False)
    degw_d = nc.declare_dram_parameter("degw", [128, NTOT], f32, False)
    gloc_d = nc.declare_dram_parameter("gloc", [128, NT], f32, False)
    W1_d = nc.declare_dram_parameter("W1b", [F, F], bf16, False)
    W2_d = nc.declare_dram_parameter("W2b", [F, F], bf16, False)
    Wmu_d = nc.declare_dram_parameter("Wmu", [F, LAT], f32, False)
    Wlv_d = nc.declare_dram_parameter("Wlv", [F, LAT], f32, False)
    b1_d = nc.declare_dram_parameter("b1c", [F, 1], f32, False)
    b2b_d = nc.declare_dram_parameter("b2b", [128, F], f32, False)
    bmub_d = nc.declare_dram_parameter("bmub", [128, LAT], f32, False)
    blvb_d = nc.declare_dram_parameter("blvb", [128, LAT], f32, False)
    iotab_d = nc.declare_dram_parameter("iotab", [128, 128], bf16, False)
    iotaf_d = nc.declare_dram_parameter("iotaf", [128, 128], f32, False)
    identf_d = nc.declare_dram_parameter("identf", [128, 128], f32, False)
    identb_d = nc.declare_dram_parameter("identb", [128, 128], bf16, False)
    icnt_d = nc.declare_dram_parameter("icnt", [128, 1], f32, False)
    mu_d = nc.declare_dram_parameter("mu_o", [128, LAT], f32, True)
    lv_d = nc.declare_dram_parameter("lv_o", [128, LAT], f32, True)

    # +128 zero rows at the tail of every gather-source tensor (identity-chunk
    # padding gathers the zero row)
    tab1w = [nc.dram_tensor(f"tab1_{w}", [WT * 128 + 128, F], bf16)
             for w in range(NW)]
    aginC = [nc.dram_tensor(f"agin_{k}", [tch[k] * 128, F], bf16)
             for k in range(NCH)]
    tab2C = [nc.dram_tensor(f"tab2_{k}", [NCORES * tch[k] * 128 + 128, F], bf16,
                            addr_space="Shared") for k in range(NCH)]

    with tile.TileContext(nc) as tc:
        with (
            tc.tile_pool(name="const", bufs=1) as cp,
            tc.tile_pool(name="accp", bufs=1) as accp,
            tc.tile_pool(name="o1p", bufs=1) as o1p,
            tc.tile_pool(name="xtp", bufs=3) as xtp,
            tc.tile_pool(name="cbp", bufs=4) as cbp,
            tc.tile_pool(name="c2p", bufs=4) as c2p,
            tc.tile_pool(name="hsp", bufs=6) as hsp,
            tc.tile_pool(name="gbp", bufs=3) as gbp,
            tc.tile_pool(name="itp", bufs=6) as itp,
            tc.tile_pool(name="ohp", bufs=6) as ohp,
            tc.tile_pool(name="ofp", bufs=3) as ofp,
            tc.tile_pool(name="o2p", bufs=3) as o2p,
            tc.tile_pool(name="psl", bufs=2, space="PSUM") as psl,
            tc.tile_pool(name="psm", bufs=1, space="PSUM") as psm,
            tc.tile_pool(name="pst", bufs=1, space="PSUM") as pst,
            tc.tile_pool(name="psg", bufs=1, space="PSUM") as psg,
        ):
            def const(d, shape, dtp, tag):
                t = cp.tile(shape, dtp, tag=tag)
                nc.sync.dma_start(out=t[:], in_=d[:, :])
                return t

            W1s = const(W1_d, [F, F], bf16, "W1s")
            W2s = const(W2_d, [F, F], bf16, "W2s")
            Wmus = const(Wmu_d, [F, LAT], f32, "Wmus")
            Wlvs = const(Wlv_d, [F, LAT], f32, "Wlvs")
            b1s = const(b1_d, [F, 1], f32, "b1s")
            b2bs = const(b2b_d, [128, F], f32, "b2bs")
            bmubs = const(bmub_d, [128, LAT], f32, "bmubs")
            blvbs = const(blvb_d, [128, LAT], f32, "blvbs")
            iotabs = const(iotab_d, [128, 128], bf16, "iotabs")
            iotafs = const(iotaf_d, [128, 128], f32, "iotafs")
            idents = const(identf_d, [128, 128], f32, "idents")
            identbs = const(identb_d, [128, 128], bf16, "identbs")
            icnts = const(icnt_d, [128, 1], f32, "icnts")

            # zero rows at the tail of every gather-source tensor
            zs = cp.tile([128, F], bf16, tag="zs")
            nc.vector.memset(zs[:], 0.0)
            for w in range(NW):
                nc.sync.dma_start(
                    out=tab1w[w][WT * 128 : WT * 128 + 128, :], in_=zs[:]
                )
            for k in range(NCH):
                r0 = NCORES * tch[k] * 128
                nc.sync.dma_start(
                    out=tab2C[k][r0 : r0 + 128, :], in_=zs[:]
                )
            dl1s = const(dl1_d, [128, CH1], bf16, "dl1s")
            dl2s = const(dl2_d, [128, CH2], bf16, "dl2s")
            glocs = const(gloc_d, [128, NT], f32, "glocs")

            dinvw = const(degw_d, [128, NTOT], f32, "dinvw")
            nc.scalar.activation(out=dinvw[:], in_=dinvw[:], func=AF.Sqrt)
            nc.vector.reciprocal(out=dinvw[:], in_=dinvw[:])
            dinvsl = dinvw  # local slice = rotated tiles 0..NT-1

            accs = {}

            # ---- Phase A: conv1 linear (bf16), full rotated table per core
            NBB = _cdiv(NTOT, 8)
            for b in range(NBB):
                j0, j1 = b * 8, min((b + 1) * 8, NTOT)
                nj = j1 - j0
                xt = xtp.tile([F, 1024], bf16, tag="xt")
                nc.scalar.dma_start(
                    out=xt[:, 0 : nj * 128],
                    in_=xT_d[:, j0 * 128 : j1 * 128],
                )
                comb = cbp.tile([128, 1024], bf16, tag="comb")
                for j in range(nj):
                    cl = j0 + j
                    ph = psl.tile([128, F], f32, tag="ph")
                    nc.tensor.matmul(
                        out=ph[:], lhsT=xt[:, j * 128 : (j + 1) * 128],
                        rhs=W1s[:], start=True, stop=True,
                    )
                    if cl < NT:
                        a = accp.tile([128, F], f32, tag=f"acc{cl}")
                        nc.scalar.activation(
                            out=a[:], in_=ph[:], func=AF.Copy,
                            scale=dinvw[:, cl : cl + 1],
                        )
                        accs[cl] = a
                        nc.vector.tensor_copy(
                            out=comb[:, j * 128 : (j + 1) * 128], in_=a[:]
                        )
                    elif cl % 2 == 0:
                        nc.scalar.activation(
                            out=comb[:, j * 128 : (j + 1) * 128], in_=ph[:],
                            func=AF.Copy, scale=dinvw[:, cl : cl + 1],
                        )
                    else:
                        nc.vector.tensor_scalar_mul(
                            out=comb[:, j * 128 : (j + 1) * 128], in0=ph[:],
                            scalar1=dinvw[:, cl : cl + 1],
                        )
                # write to window tensors, splitting at window boundaries
                j = 0
                while j < nj:
                    cl = j0 + j
                    w = cl // WT
                    je = j + 1
                    while je < nj and (j0 + je) // WT == w:
                        je += 1
                    r0 = (cl - w * WT) * 128
                    nseg = je - j
                    nc.sync.dma_start(
                        out=tab1w[w][r0 : r0 + nseg * 128, :].rearrange(
                            "(q p) f -> p q f", p=128),
                        in_=comb[:, j * 128 : je * 128].rearrange(
                            "p (q f) -> p q f", f=128),
                    )
                    j = je

            # ---- shared message-passing pass ----
            # Per-tile PSUM chains stay open across all windows of a block;
            # one merge per tile per conv.
            def conv_mp(tabsrcs, Qid, Qoh, offs, calls, cmap, dls, idx_d,
                        out_cb):
                NWIN = len(tabsrcs)
                Qtot = Qid + Qoh
                wfirst = {}
                wlast = {}
                for t in range(NT):
                    ws = [w for w in range(NWIN) if Qtot[w][t] > 0]
                    if ws:
                        wfirst[t], wlast[t] = ws[0], ws[-1]
                qsel = 0
                for b in range(NBLKP):
                    t0, t1 = b * TCHP, min((b + 1) * TCHP, NT)
                    pms = {}
                    for w in range(NWIN):
                        gt = {}
                        for (bb, ww, c0, c1) in calls:
                            if bb != b or ww != w:
                                continue
                            nb_ = c1 - c0
                            it = itp.tile([128, GC * 8], i16, tag="it")
                            nc.scalar.dma_start(
                                out=it[:, 0 : nb_ * 8],
                                in_=idx_d[:, c0 * 8 : c1 * 8],
                            )
                            gb = gbp.tile([128, GC, 128], bf16, tag="gb")
                            nc.gpsimd.dma_gather(
                                gb[:, 0:nb_, :],
                                tabsrcs[w][:, :],
                                it[:, 0 : nb_ * 8],
                                nb_ * 128, nb_ * 128, 128,
                                single_packet=False,
                                queue_num=qsel,
                            )
                            qsel ^= 1
                            gt[c0] = gb
                        for t in range(t0, t1):
                            nid = int(Qid[w][t])
                            qb = int(Qoh[w][t])
                            qn = nid + qb
                            if qn == 0:
                                continue
                            g0 = int(offs[w][t])
                            if qb > 0:
                                oh = ohp.tile([128, QMAX * 128], bf16, tag="oh")
                                nc.vector.tensor_tensor(
                                    out=oh[:, 0 : qb * 128].rearrange(
                                        "p (q i) -> p q i", i=128),
                                    in0=dls[:, g0 + nid : g0 + qn].unsqueeze(2)
                                        .to_broadcast([128, qb, 128]),
                                    in1=iotabs[:].unsqueeze(1)
                                        .to_broadcast([128, qb, 128]),
                                    op=OP.is_equal,
                                )
                            if t not in pms:
                                pms[t] = psm.tile(
                                    [128, F], f32,
                                    name=f"pm{t - t0}", tag=f"pm{t - t0}")
                            pm = pms[t]
                            for k in range(qn):
                                q = g0 + k
                                ci, cj = cmap[q]
                                lhsT = (identbs[:] if k < nid else
                                        oh[:, (k - nid) * 128 : (k - nid + 1) * 128])
                                nc.tensor.matmul(
                                    out=pm[:],
                                    lhsT=lhsT,
                                    rhs=gt[ci][:, cj, 0:128],
                                    start=(w == wfirst[t] and k == 0),
                                    stop=(w == wlast[t] and k == qn - 1),
                                    skip_group_check=True,
                                )
                    for t in range(t0, t1):
                        if t in pms:
                            nc.vector.tensor_add(
                                out=accs[t][:], in0=accs[t][:], in1=pms[t][:]
                            )
                    out_cb(b, t0, t1)

            # ---- conv1 per-block tail: relu, transpose, conv2 linear, AG
            def cb1(b, t0, t1):
                for t in range(t0, t1):
                    hs = hsp.tile([128, F], f32, tag="hs")
                    nc.scalar.activation(
                        out=hs[:], in_=accs[t][:], func=AF.Copy,
                        scale=dinvsl[:, t : t + 1],
                    )
                    ptr = pst.tile([128, 128], f32, tag="ptr")
                    nc.tensor.transpose(out=ptr[:], in_=hs[:], identity=idents[:])
                    o1 = o1p.tile([128, 128], bf16, tag=f"o1T{t}")
                    nc.scalar.activation(
                        out=o1[:], in_=ptr[:], func=AF.Relu, bias=b1s[:, 0:1]
                    )
                    # conv2 linear on this tile
                    ph2 = psl.tile([128, F], f32, tag="ph")
                    nc.tensor.matmul(
                        out=ph2[:], lhsT=o1[:], rhs=W2s[:], start=True, stop=True
                    )
                    a2 = accp.tile([128, F], f32, tag=f"acc{t}")
                    nc.scalar.activation(
                        out=a2[:], in_=ph2[:], func=AF.Copy,
                        scale=dinvsl[:, t : t + 1],
                    )
                    accs[t] = a2
                    c2 = c2p.tile([128, F], bf16, tag="c2")
                    nc.vector.tensor_copy(out=c2[:], in_=a2[:])
                    k = int(np.searchsorted(cstart, t, side="right")) - 1
                    rt = t - int(cstart[k])
                    nc.sync.dma_start(
                        out=aginC[k][rt * 128 : (rt + 1) * 128, :], in_=c2[:]
                    )
                if stop_after is None or stop_after == "C":
                    # fire each AllGather chunk once its tiles are all written
                    for k in range(NCH):
                        lastt = int(cstart[k + 1]) - 1
                        if t0 <= lastt < t1:
                            nc.gpsimd.collective_compute(
                                "AllGather", OP.bypass,
                                replica_groups=[list(range(NCORES))],
                                ins=[aginC[k].ap().opt()],
                                outs=[tab2C[k][0 : NCORES * tch[k] * 128, :].opt()],
                            )

            # ---- conv2 per-tile tail: relu + pooling accumulate
            gps = psg.tile([128, F], f32, tag="gps")

            def cb2(t, first, last):
                o2 = o2p.tile([128, F], f32, tag="o2")
                nc.scalar.activation(
                    out=o2[:], in_=accs[t][:], func=AF.Copy,
                    scale=dinvsl[:, t : t + 1],
                )
                nc.vector.tensor_add(out=o2[:], in0=o2[:], in1=b2bs[:])
                nc.vector.tensor_relu(out=o2[:], in_=o2[:])
                ohf = ofp.tile([128, 128], f32, tag="ohf")
                nc.vector.tensor_tensor(
                    out=ohf[:],
                    in0=glocs[:, t : t + 1].to_broadcast([128, 128]),
                    in1=iotafs[:], op=OP.is_equal,
                )
                nc.tensor.matmul(
                    out=gps[:], lhsT=ohf[:], rhs=o2[:],
                    start=first, stop=last,
                    skip_group_check=True,
                )

            def dump_rows(dram_rows, od):
                tt = hsp.tile([128, F], bf16, tag="dbgb")
                nc.sync.dma_start(out=tt[:], in_=dram_rows)
                dd = hsp.tile([128, LAT], f32, tag="ms")
                nc.vector.tensor_copy(out=dd[:], in_=tt[:, 0:LAT])
                nc.sync.dma_start(out=od[:, :], in_=dd[:])

            def dump_sbuf(st, od):
                dd = hsp.tile([128, LAT], f32, tag="ms")
                nc.vector.tensor_copy(out=dd[:], in_=st[:, 0:LAT])
                nc.sync.dma_start(out=od[:, :], in_=dd[:])

            if stop_after == "A":
                dump_rows(tab1w[0][0:128, :], mu_d)
                dump_rows(tab1w[NW - 1][(WT - 1) * 128 : WT * 128, :], lv_d)
            if stop_after is None or stop_after in ("B", "C"):
                conv_mp(tab1w, s1["Qid"], s1["Qoh"], s1["offs"], s1["calls"],
                        cmap1, dl1s, idx1_d, cb1)
            if stop_after == "B":
                dump_sbuf(o1p.tile([128, 128], bf16, tag="o1T0"), mu_d)
                dump_sbuf(o1p.tile([128, 128], bf16, tag=f"o1T{NT - 1}"), lv_d)
            if stop_after == "C":
                dump_rows(tab2C[0][0:128, :], mu_d)
                dump_rows(tab2C[NCH - 1][127 * 128 : 128 * 128, :], lv_d)

            if stop_after is None:
                # ---- conv2 mp: window-major, per-(window,tile) PSUM chains
                Qid2, Qoh2, offs2 = s2["Qid"], s2["Qoh"], s2["offs"]
                wlast2 = {}
                for t in range(NT):
                    ws = [w for w in range(NCH) if Qid2[w][t] + Qoh2[w][t] > 0]
                    wlast2[t] = ws[-1] if ws else -1
                ndone = 0
                for t in range(NT):
                    if wlast2[t] < 0:  # no messages at all: pool the init acc
                        cb2(t, ndone == 0, ndone == NT - 1)
                        ndone += 1
                qsel2 = 0
                g = 0
                for w in range(NCH):
                    gt = {}
                    for (bb, ww, c0, c1) in s2["calls"]:
                        if ww != w:
                            continue
                        nb_ = c1 - c0
                        it = itp.tile([128, GC * 8], i16, tag="it")
                        nc.scalar.dma_start(
                            out=it[:, 0 : nb_ * 8],
                            in_=idx2_d[:, c0 * 8 : c1 * 8],
                        )
                        gb = gbp.tile([128, GC, 128], bf16, tag="gb")
                        nc.gpsimd.dma_gather(
                            gb[:, 0:nb_, :],
                            tab2C[w][:, :],
                            it[:, 0 : nb_ * 8],
                            nb_ * 128, nb_ * 128, 128,
                            single_packet=False,
                            queue_num=qsel2,
                        )
                        qsel2 ^= 1
                        gt[c0] = gb
                    for t in range(NT):
                        nid = int(Qid2[w][t])
                        qb = int(Qoh2[w][t])
                        qn = nid + qb
                        if qn == 0:
                            continue
                        g0 = int(offs2[w][t])
                        if qb > 0:
                            oh = ohp.tile([128, QMAX * 128], bf16, tag="oh")
                            nc.vector.tensor_tensor(
                                out=oh[:, 0 : qb * 128].rearrange(
                                    "p (q i) -> p q i", i=128),
                                in0=dl2s[:, g0 + nid : g0 + qn].unsqueeze(2)
                                    .to_broadcast([128, qb, 128]),
                                in1=iotabs[:].unsqueeze(1)
                                    .to_broadcast([128, qb, 128]),
                                op=OP.is_equal,
                            )
                        j = g % 4
                        g += 1
                        pm = psm.tile([128, F], f32,
                                      name=f"pm{j}", tag=f"pm{j}")
                        for k in range(qn):
                            ci, cj = cmap2[g0 + k]
                            lhsT = (identbs[:] if k < nid else
                                    oh[:, (k - nid) * 128 : (k - nid + 1) * 128])
                            nc.tensor.matmul(
                                out=pm[:],
                                lhsT=lhsT,
                                rhs=gt[ci][:, cj, 0:128],
                                start=(k == 0), stop=(k == qn - 1),
                                skip_group_check=True,
                            )
                        nc.vector.tensor_add(
                            out=accs[t][:], in0=accs[t][:], in1=pm[:]
                        )
                        if w == wlast2[t]:
                            cb2(t, ndone == 0, ndone == NT - 1)
                            ndone += 1

                # ---- mean + heads
                gm = hsp.tile([128, F], f32, tag="gm")
                nc.scalar.activation(
                    out=gm[:], in_=gps[:], func=AF.Copy, scale=icnts[:, 0:1]
                )
                pgt = pst.tile([128, 128], f32, tag="ptr")
                nc.tensor.transpose(out=pgt[:], in_=gm[:], identity=idents[:])
                gT = hsp.tile([128, 128], f32, tag="gT")
                nc.vector.tensor_copy(out=gT[:], in_=pgt[:])
                for Wd, bb, od in ((Wmus, bmubs, mu_d), (Wlvs, blvbs, lv_d)):
                    pmu = psl.tile([128, LAT], f32, tag="ph")
                    nc.tensor.matmul(
                        out=pmu[:], lhsT=gT[:], rhs=Wd[:], start=True, stop=True
                    )
                    ms = hsp.tile([128, LAT], f32, tag="ms")
                    nc.vector.tensor_add(out=ms[:], in0=pmu[:], in1=bb[:])
                    nc.sync.dma_start(out=od[:, :], in_=ms[:])

    nc.compile()
    return nc


def make_in_maps(pr, W1, b1, W2, b2, Wmu, bmu, Wlv, blv):
    F = pr["F"]
    iota = np.tile(np.arange(128, dtype=np.float32), (128, 1))
    shared = {
        "W1b": np.asarray(W1, np.float32).astype(BF16),
        "W2b": np.asarray(W2, np.float32).astype(BF16),
        "Wmu": np.asarray(Wmu, np.float32), "Wlv": np.asarray(Wlv, np.float32),
        "b1c": np.asarray(b1, np.float32).reshape(F, 1),
        "b2b": np.tile(np.asarray(b2, np.float32), (128, 1)),
        "bmub": np.tile(np.asarray(bmu, np.float32), (128, 1)),
        "blvb": np.tile(np.asarray(blv, np.float32), (128, 1)),
        "iotab": iota.astype(BF16), "iotaf": iota,
        "identf": np.eye(128, dtype=np.float32),
        "identb": np.eye(128, dtype=np.float32).astype(BF16),
    }
    maps = []
    for c in range(NCORES):
        m = dict(shared)
        m["xT"] = pr["xTs"][c]
        m["degw"] = pr["degws"][c]
        m["idx1"] = pr["s1"]["idx16"][c]
        m["dl1"] = pr["s1"]["dl"][c]
        m["idx2"] = pr["s2"]["idx16"][c]
        m["dl2"] = pr["s2"]["dl"][c]
        m["gloc"] = pr["glocw"][c]
        m["icnt"] = pr["icnt"][c]
        maps.append(m)
    return maps


def kernel(x, edge_index, batch, W1, b1, W2, b2, Wmu, bmu, Wlv, blv):
    n_graphs = 1024
    pr = prep_host(x, edge_index, batch, n_graphs)
    LAT = np.asarray(Wmu).shape[1]
    nc = build_nc(pr, LAT)
    in_maps = make_in_maps(pr, W1, b1, W2, b2, Wmu, bmu, Wlv, blv)
    res = run_bass_kernel_spmd(nc, in_maps, core_ids=list(range(NCORES)))
    mu = np.concatenate([res.results[c]["mu_o"] for c in range(NCORES)], axis=0)
    lv = np.concatenate([res.results[c]["lv_o"] for c in range(NCORES)], axis=0)
    return (mu.astype(np.float32), lv.astype(np.float32))


# revision 70
# speedup vs baseline: 1.0050x; 1.0050x over previous
"""GCN encoder (2x GCNConv + global_mean_pool + two linear heads) on 8 trn2 cores.

Strategy (SPMD, one program, per-core data):
  - 1024 graphs -> 128 graphs/core; nodes re-indexed into a padded per-core
    layout (SLICE = NT*128 rows/core, PN = 8*SLICE).
  - Tables stored as single bf16 [rows, 128] (256B rows, min dma_gather
    granularity).  Message passing gathers h~[src] rows with dma_gather
    (dst-sorted edge chunks of 128, int16 indices relative to a <=32K-row
    window), builds one-hot routing matrices (batched is_equal vs iota) and
    accumulates onehot.T @ gathered into PSUM per (window, dst-tile) group,
    merged into SBUF f32 accumulators.
  - conv1 uses a per-core ROTATED layout (own slice first): Phase A computes
    x@W1 (bf16) for the full table redundantly per core; local tiles also
    init the SBUF accumulators (covers the self-loop term, no indirect DMA).
    tab1 is split into NW window tensors so gathers overlap Phase A's tail.
  - conv2: per dst-tile after conv1 mp: relu -> transpose -> @W2 -> local
    h2~ tiles (acc init for conv2) + bf16 copies AllGathered chunk-by-chunk
    (NCH=4 chunked collectives, overlapped with conv1 mp of later blocks).
    conv2 mp is window-major (chunk-tensor-major) so windows 0..2 hide the
    tail AllGather; conv1 mp is block-major (TCHP tiles/block, per-tile PSUM
    chain across windows; each open chain owns a full PSUM bank because
    start=True clears has_written bank-wide).
  - Most messages ride IDENTITY chunks (slot==dst-local, constant identity
    lhsT, no one-hot build); only per-dst overflow edges get DVE-built
    one-hots.  Identity padding gathers a dedicated zero row.
  - Pooling: one-hot by graph-local id, matmul accumulate; per-graph counts
    are host-precomputed.  Outputs per core: mu/logvar for its 128 graphs.
"""

import numpy as np
import ml_dtypes

import concourse.bass as bass
import concourse.bacc as bacc
import concourse.mybir as mybir
import concourse.tile as tile
from concourse.bass_utils import run_bass_kernel_spmd

BF16 = ml_dtypes.bfloat16
NCORES = 8
NW = 4  # conv1 gather windows (window = 2*SLICE rows, fits int16)
NCH = 4  # conv2 chunk tensors == AllGather chunks == processing blocks
PAD_DL = 200.0  # one-hot miss marker (exact in bf16, outside 0..127)
GC = 64  # max chunks (of 128 gathered rows) per dma_gather call
LID = 5  # identity chunks per (window, tile) group
TCHP = 4  # tiles per processing block (each open PSUM chain owns a bank:
          # start=True clears has_written for the WHOLE bank, so interleaved
          # chains must never share one)


def _cdiv(a, b):
    return -(-a // b)


def _build_stream(owner, w, srel, lt, dloc, NWIN, NT, TCH, L, zrows):
    """Edge stream in (block, window, tile) order with shared chunk quotas.

    Per (window, tile) group: the first min(L, .) messages of every dst node
    go to IDENTITY chunks (slot == dst-local, no one-hot needed; short nodes
    pad with the window's zero row); overflow edges pack densely into one-hot
    chunks.  Returns Qid/Qoh [NWIN, NT], offs, calls, CH, dl, idx16."""
    E = owner.shape[0]
    # per-dst counts within each (owner, window, tile) group
    cnt4 = np.zeros((NCORES, NWIN, NT, 128), np.int32)
    np.add.at(cnt4, (owner, w, lt, dloc), 1)
    maxn = cnt4.max(axis=3).max(axis=0)  # [NWIN, NT]
    # pick identity level per window: minimize gather chunks + ~0.7x the
    # one-hot chunks (their extra DVE build cost relative to a chunk's DMA)
    Lw = np.zeros(NWIN, np.int64)
    for w_ in range(NWIN):
        best = None
        for Lc in range(0, L + 3):
            qid = np.minimum(Lc, maxn[w_])
            over = np.maximum(cnt4[:, w_] - Lc, 0).sum(axis=2)
            qoh = _cdiv(over.max(axis=0), 128)
            cost = (qid + qoh).sum() + 0.25 * qoh.sum()
            if best is None or cost < best[0]:
                best = (cost, Lc)
        Lw[w_] = best[1]
    Qid = np.minimum(Lw[:, None], maxn).astype(np.int64)
    Lpere = Lw[w]  # per-edge identity level
    nover = np.maximum(cnt4 - Lw[None, :, None, None], 0).sum(axis=3)
    Qoh = _cdiv(nover.max(axis=0), 128).astype(np.int64)
    Q = Qid + Qoh

    NBLK = _cdiv(NT, TCH)
    offs = np.zeros((NWIN, NT), np.int64)
    calls = []
    chunk_w = []
    c = 0
    for b in range(NBLK):
        t0, t1 = b * TCH, min((b + 1) * TCH, NT)
        for w_ in range(NWIN):
            span0 = c
            for t in range(t0, t1):
                offs[w_, t] = c
                c += int(Q[w_, t])
                chunk_w.extend([w_] * int(Q[w_, t]))
            n = c - span0
            ncalls = _cdiv(n, GC)
            cc = span0
            for i in range(ncalls):
                sz = n // ncalls + (1 if i < n % ncalls else 0)
                calls.append((b, w_, cc, cc + sz))
                cc += sz
    CH = c

    blk = lt // TCH
    order = np.lexsort((dloc, lt, w, blk, owner))
    so, sw, st = owner[order], w[order], lt[order]
    sr = srel[order].astype(np.int64)
    sd = dloc[order]
    # rank within (owner, w, t, dst)
    grpd = ((so * NWIN + sw) * NT + st) * 128 + sd
    start_of = np.zeros(E, np.int64)
    is_new = np.ones(E, bool)
    is_new[1:] = grpd[1:] != grpd[:-1]
    start_of[is_new] = np.arange(E)[is_new]
    start_of = np.maximum.accumulate(start_of)
    r = np.arange(E) - start_of

    pos = np.empty(E, np.int64)
    sel1 = r < Lpere[order]
    pos[sel1] = (offs[sw[sel1], st[sel1]] + r[sel1]) * 128 + sd[sel1]
    # overflow edges: dense rank within (owner, w, t)
    i2 = np.flatnonzero(~sel1)
    grp2 = (so[i2] * NWIN + sw[i2]) * NT + st[i2]
    st2 = np.zeros(len(i2), np.int64)
    isn2 = np.ones(len(i2), bool)
    isn2[1:] = grp2[1:] != grp2[:-1]
    st2[isn2] = np.arange(len(i2))[isn2]
    st2 = np.maximum.accumulate(st2)
    r2 = np.arange(len(i2)) - st2
    pos[i2] = (offs[sw[i2], st[i2]] + Qid[sw[i2], st[i2]]) * 128 + r2

    dl = np.full((NCORES, 128, CH), PAD_DL, np.float32)
    dl[so[i2], pos[i2] % 128, pos[i2] // 128] = sd[i2]
    # default index = the window's zero row (identity-chunk padding must
    # contribute zero); one-hot padding also lands there harmlessly
    zc = np.asarray(zrows, np.int16)[np.asarray(chunk_w, np.int64)]  # [CH]
    V = np.tile(np.repeat(zc, 128)[None, :], (NCORES, 1))
    V[so, pos] = sr
    idx16 = np.zeros((NCORES, 128, CH * 8), np.int16)
    for cc in range(NCORES):
        for (_, _, c0, c1) in calls:
            b_ = V[cc, c0 * 128 : c1 * 128].reshape(-1, 16).T  # [16, ncols]
            for g in range(8):
                idx16[cc, g * 16 : (g + 1) * 16, c0 * 8 : c1 * 8] = b_
    return dict(Qid=Qid, Qoh=Qoh, Q=Q, offs=offs, calls=calls, CH=CH,
                dl=dl.astype(BF16), idx16=idx16)


def prep_host(x, edge_index, batch, n_graphs):
    x = np.asarray(x, np.float32)
    edge_index = np.asarray(edge_index)
    batch = np.asarray(batch).astype(np.int64)
    N, F = x.shape
    gpc = n_graphs // NCORES

    core_of_node = (batch // gpc).astype(np.int64)  # sorted non-decreasing
    counts = np.bincount(core_of_node, minlength=NCORES)
    starts = np.zeros(NCORES + 1, np.int64)
    starts[1:] = np.cumsum(counts)
    NT = int(_cdiv(int(counts.max()), 128))
    SLICE = NT * 128
    assert 2 * SLICE <= 32767, "window exceeds int16 range"
    PN = NCORES * SLICE
    NTOT = PN // 128
    TCH = _cdiv(NT, NCH)
    tch = [min(TCH, NT - k * TCH) for k in range(NCH)]
    assert all(t > 0 for t in tch), f"bad chunking {tch}"
    assert max(tch) * 128 * NCORES + 128 <= 32767, "chunk tensor exceeds int16"
    cstart = np.zeros(NCH + 1, np.int64)
    cstart[1:] = np.cumsum(tch)

    rank = np.empty(N, np.int64)
    for c in range(NCORES):
        rank[starts[c] : starts[c + 1]] = np.arange(counts[c])
    tile_of = rank // 128
    part_of = rank % 128

    src = edge_index[0].astype(np.int64)
    dst = edge_index[1].astype(np.int64)
    deg = np.bincount(dst, minlength=N).astype(np.float32) + 1.0  # + self loop

    owner = core_of_node[dst]
    lt = tile_of[dst]
    dloc = part_of[dst]

    # conv1: rotated layout (own slice first); window = 2 consecutive slices
    rel = (core_of_node[src] - owner) % NCORES
    w1 = rel >> 1
    srel1 = (rel & 1) * SLICE + rank[src]
    s1 = _build_stream(owner, w1, srel1, lt, dloc, NW, NT, TCHP,
                       LID, [2 * SLICE] * NW)

    # conv2: standard layout, chunk tensors over tile ranges
    k2 = np.searchsorted(cstart, tile_of[src], side="right") - 1
    tch_arr = np.asarray(tch, np.int64)
    srel2 = (core_of_node[src] * (tch_arr[k2] * 128)
             + (rank[src] - cstart[k2] * 128))
    # conv2 stream is window-major (single block): windows 0..2 are ready
    # early, so conv2 work hides the tail AllGather
    s2 = _build_stream(owner, k2, srel2, lt, dloc, NCH, NT, NT,
                       LID, [NCORES * t * 128 for t in tch])

    # standard padded transposed features + degree table, then per-core rotate.
    # x is pre-scaled by dinv so Phase A needs no per-tile output scaling:
    # dinv .* (x @ W1) == (dinv .* x) @ W1
    xT_std = np.zeros((F, PN), BF16)
    pid = core_of_node * SLICE + rank
    dinv_node = 1.0 / np.sqrt(deg)
    xT_std[:, pid] = (x * dinv_node[:, None]).T.astype(BF16)
    degw_std = np.ones((128, NTOT), np.float32)
    degw_std[part_of, pid // 128] = deg
    xTs = np.stack([
        np.concatenate([xT_std[:, c * SLICE :], xT_std[:, : c * SLICE]], axis=1)
        for c in range(NCORES)
    ])
    degws = np.stack([
        np.concatenate([degw_std[:, c * NT :], degw_std[:, : c * NT]], axis=1)
        for c in range(NCORES)
    ])

    glocw = np.full((NCORES, 128, NT), PAD_DL, np.float32)
    glocw[core_of_node, part_of, tile_of] = (batch % gpc).astype(np.float32)

    # per-graph node counts (for the pooling mean) are host-derivable
    gcnt = np.bincount(batch, minlength=n_graphs).astype(np.float32)
    icnt = (1.0 / np.maximum(gcnt, 1.0)).reshape(NCORES, 128, 1)

    return dict(NT=NT, PN=PN, F=F, tch=tch, cstart=cstart, s1=s1, s2=s2,
                xTs=xTs, degws=degws, glocw=glocw, icnt=icnt)


def build_nc(pr, LAT, stop_after=None):
    dt = mybir.dt
    f32, bf16, i16 = dt.float32, dt.bfloat16, dt.int16
    NT, PN, F = pr["NT"], pr["PN"], pr["F"]
    tch, cstart = pr["tch"], pr["cstart"]
    NTOT = PN // 128
    NB = PN // 512
    WT = (PN // NW) // 128  # tiles per conv1 window tensor
    s1, s2 = pr["s1"], pr["s2"]
    CH1, CH2 = s1["CH"], s2["CH"]
    QMAX = int(max(s1["Qoh"].max(), s2["Qoh"].max(), 1))
    NBLKP = _cdiv(NT, TCHP)

    def _call_map(calls):
        m = {}
        for (_, _, c0, c1) in calls:
            for q in range(c0, c1):
                m[q] = (c0, q - c0)
        return m

    cmap1, cmap2 = _call_map(s1["calls"]), _call_map(s2["calls"])
    AF = mybir.ActivationFunctionType
    OP = mybir.AluOpType

    nc = bacc.Bacc(num_swdge_queues=2)
    xT_d = nc.declare_dram_parameter("xT", [F, PN], bf16, False)
    idx1_d = nc.declare_dram_parameter("idx1", [128, CH1 * 8], i16, False)
    dl1_d = nc.declare_dram_parameter("dl1", [128, CH1], bf16, False)
    idx2_d = nc.declare_dram_parameter("idx2", [128, CH2 * 8], i16, False)
    dl2_d = nc.declare_dram_parameter("dl2", [128, CH2], bf16, False)
    degw_d = nc.declare_dram_parameter("degw", [128, NTOT], f32, False)
    gloc_d = nc.declare_dram_parameter("gloc", [128, NT], f32, False)
    W1_d = nc.declare_dram_parameter("W1b", [F, F], bf16, False)
    W2_d = nc.declare_dram_parameter("W2b", [F, F], bf16, False)
    Wmu_d = nc.declare_dram_parameter("Wmu", [F, LAT], f32, False)
    Wlv_d = nc.declare_dram_parameter("Wlv", [F, LAT], f32, False)
    b1_d = nc.declare_dram_parameter("b1c", [F, 1], f32, False)
    b2b_d = nc.declare_dram_parameter("b2b", [128, F], f32, False)
    bmub_d = nc.declare_dram_parameter("bmub", [128, LAT], f32, False)
    blvb_d = nc.declare_dram_parameter("blvb", [128, LAT], f32, False)
    iotab_d = nc.declare_dram_parameter("iotab", [128, 128], bf16, False)
    iotaf_d = nc.declare_dram_parameter("iotaf", [128, 128], f32, False)
    identf_d = nc.declare_dram_parameter("identf", [128, 128], f32, False)
    identb_d = nc.declare_dram_parameter("identb", [128, 128], bf16, False)
    icnt_d = nc.declare_dram_parameter("icnt", [128, 1], f32, False)
    mu_d = nc.declare_dram_parameter("mu_o", [128, LAT], f32, True)
    lv_d = nc.declare_dram_parameter("lv_o", [128, LAT], f32, True)

    # +128 zero rows at the tail of every gather-source tensor (identity-chunk
    # padding gathers the zero row)
    tab1w = [nc.dram_tensor(f"tab1_{w}", [WT * 128 + 128, F], bf16)
             for w in range(NW)]
    aginC = [nc.dram_tensor(f"agin_{k}", [tch[k] * 128, F], bf16)
             for k in range(NCH)]
    tab2C = [nc.dram_tensor(f"tab2_{k}", [NCORES * tch[k] * 128 + 128, F], bf16,
                            addr_space="Shared") for k in range(NCH)]

    with tile.TileContext(nc) as tc:
        with (
            tc.tile_pool(name="const", bufs=1) as cp,
            tc.tile_pool(name="accp", bufs=1) as accp,
            tc.tile_pool(name="o1p", bufs=1) as o1p,
            tc.tile_pool(name="xtp", bufs=3) as xtp,
            tc.tile_pool(name="cbp", bufs=4) as cbp,
            tc.tile_pool(name="c2p", bufs=4) as c2p,
            tc.tile_pool(name="hsp", bufs=6) as hsp,
            tc.tile_pool(name="gbp", bufs=3) as gbp,
            tc.tile_pool(name="itp", bufs=6) as itp,
            tc.tile_pool(name="ohp", bufs=6) as ohp,
            tc.tile_pool(name="ofp", bufs=4) as ofp,
            tc.tile_pool(name="o2p", bufs=4) as o2p,
            tc.tile_pool(name="psl", bufs=2, space="PSUM") as psl,
            tc.tile_pool(name="psm", bufs=1, space="PSUM") as psm,
            tc.tile_pool(name="pst", bufs=1, space="PSUM") as pst,
            tc.tile_pool(name="psg", bufs=1, space="PSUM") as psg,
        ):
            def const(d, shape, dtp, tag):
                t = cp.tile(shape, dtp, tag=tag)
                nc.sync.dma_start(out=t[:], in_=d[:, :])
                return t

            W1s = const(W1_d, [F, F], bf16, "W1s")
            W2s = const(W2_d, [F, F], bf16, "W2s")
            Wmus = const(Wmu_d, [F, LAT], f32, "Wmus")
            Wlvs = const(Wlv_d, [F, LAT], f32, "Wlvs")
            b1s = const(b1_d, [F, 1], f32, "b1s")
            b2bs = const(b2b_d, [128, F], f32, "b2bs")
            bmubs = const(bmub_d, [128, LAT], f32, "bmubs")
            blvbs = const(blvb_d, [128, LAT], f32, "blvbs")
            iotabs = const(iotab_d, [128, 128], bf16, "iotabs")
            iotafs = const(iotaf_d, [128, 128], f32, "iotafs")
            idents = const(identf_d, [128, 128], f32, "idents")
            identbs = const(identb_d, [128, 128], bf16, "identbs")
            icnts = const(icnt_d, [128, 1], f32, "icnts")

            # zero rows at the tail of every gather-source tensor
            zs = cp.tile([128, F], bf16, tag="zs")
            nc.vector.memset(zs[:], 0.0)
            for w in range(NW):
                nc.sync.dma_start(
                    out=tab1w[w][WT * 128 : WT * 128 + 128, :], in_=zs[:]
                )
            for k in range(NCH):
                r0 = NCORES * tch[k] * 128
                nc.sync.dma_start(
                    out=tab2C[k][r0 : r0 + 128, :], in_=zs[:]
                )
            dl1s = const(dl1_d, [128, CH1], bf16, "dl1s")
            dl2s = const(dl2_d, [128, CH2], bf16, "dl2s")
            glocs = const(gloc_d, [128, NT], f32, "glocs")

            dinvw = const(degw_d, [128, NTOT], f32, "dinvw")
            nc.scalar.activation(out=dinvw[:], in_=dinvw[:], func=AF.Sqrt)
            nc.vector.reciprocal(out=dinvw[:], in_=dinvw[:])
            dinvsl = dinvw  # local slice = rotated tiles 0..NT-1

            accs = {}

            # ---- Phase A: conv1 linear (bf16), full rotated table per core
            NBB = _cdiv(NTOT, 8)
            for b in range(NBB):
                j0, j1 = b * 8, min((b + 1) * 8, NTOT)
                nj = j1 - j0
                xt = xtp.tile([F, 1024], bf16, tag="xt")
                nc.scalar.dma_start(
                    out=xt[:, 0 : nj * 128],
                    in_=xT_d[:, j0 * 128 : j1 * 128],
                )
                comb = cbp.tile([128, 1024], bf16, tag="comb")
                for h in range(_cdiv(nj, 4)):
                    ja, jb = h * 4, min(h * 4 + 4, nj)
                    pb = psl.tile([128, 512], f32, tag="ph")
                    for j in range(ja, jb):
                        # single-mm groups sharing a bank: each start clears
                        # only has_written bits; prior slices' DATA is intact
                        nc.tensor.matmul(
                            out=pb[:, (j - ja) * 128 : (j - ja + 1) * 128],
                            lhsT=xt[:, j * 128 : (j + 1) * 128],
                            rhs=W1s[:], start=True, stop=True,
                            skip_group_check=True,
                        )
                    if j0 + jb <= NT:  # fully local: per-tile f32 accs
                        for j in range(ja, jb):
                            cl = j0 + j
                            a = accp.tile([128, F], f32, tag=f"acc{cl}")
                            nc.vector.tensor_copy(
                                out=a[:],
                                in_=pb[:, (j - ja) * 128 : (j - ja + 1) * 128],
                            )
                            accs[cl] = a
                        nc.scalar.activation(
                            out=comb[:, ja * 128 : jb * 128],
                            in_=pb[:, 0 : (jb - ja) * 128], func=AF.Copy,
                        )
                    elif j0 + ja >= NT:  # fully non-local: one wide drain
                        if h % 2 == 0:
                            nc.scalar.activation(
                                out=comb[:, ja * 128 : jb * 128],
                                in_=pb[:, 0 : (jb - ja) * 128], func=AF.Copy,
                            )
                        else:
                            nc.vector.tensor_copy(
                                out=comb[:, ja * 128 : jb * 128],
                                in_=pb[:, 0 : (jb - ja) * 128],
                            )
                    else:  # straddles the local boundary
                        for j in range(ja, jb):
                            cl = j0 + j
                            sl = pb[:, (j - ja) * 128 : (j - ja + 1) * 128]
                            if cl < NT:
                                a = accp.tile([128, F], f32, tag=f"acc{cl}")
                                nc.vector.tensor_copy(out=a[:], in_=sl)
                                accs[cl] = a
                            nc.scalar.activation(
                                out=comb[:, j * 128 : (j + 1) * 128],
                                in_=sl, func=AF.Copy,
                            )
                # write to window tensors, splitting at window boundaries
                j = 0
                while j < nj:
                    cl = j0 + j
                    w = cl // WT
                    je = j + 1
                    while je < nj and (j0 + je) // WT == w:
                        je += 1
                    r0 = (cl - w * WT) * 128
                    nseg = je - j
                    nc.sync.dma_start(
                        out=tab1w[w][r0 : r0 + nseg * 128, :].rearrange(
                            "(q p) f -> p q f", p=128),
                        in_=comb[:, j * 128 : je * 128].rearrange(
                            "p (q f) -> p q f", f=128),
                    )
                    j = je

            # ---- shared message-passing pass ----
            # Per-tile PSUM chains stay open across all windows of a block;
            # one merge per tile per conv.
            def conv_mp(tabsrcs, Qid, Qoh, offs, calls, cmap, dls, idx_d,
                        out_cb):
                NWIN = len(tabsrcs)
                Qtot = Qid + Qoh
                wfirst = {}
                wlast = {}
                for t in range(NT):
                    ws = [w for w in range(NWIN) if Qtot[w][t] > 0]
                    if ws:
                        wfirst[t], wlast[t] = ws[0], ws[-1]
                qsel = 0
                for b in range(NBLKP):
                    t0, t1 = b * TCHP, min((b + 1) * TCHP, NT)
                    pms = {}
                    for w in range(NWIN):
                        gt = {}
                        for (bb, ww, c0, c1) in calls:
                            if bb != b or ww != w:
                                continue
                            nb_ = c1 - c0
                            it = itp.tile([128, GC * 8], i16, tag="it")
                            nc.scalar.dma_start(
                                out=it[:, 0 : nb_ * 8],
                                in_=idx_d[:, c0 * 8 : c1 * 8],
                            )
                            gb = gbp.tile([128, GC, 128], bf16, tag="gb")
                            nc.gpsimd.dma_gather(
                                gb[:, 0:nb_, :],
                                tabsrcs[w][:, :],
                                it[:, 0 : nb_ * 8],
                                nb_ * 128, nb_ * 128, 128,
                                single_packet=False,
                                queue_num=qsel,
                            )
                            qsel ^= 1
                            gt[c0] = gb
                        for t in range(t0, t1):
                            nid = int(Qid[w][t])
                            qb = int(Qoh[w][t])
                            qn = nid + qb
                            if qn == 0:
                                continue
                            g0 = int(offs[w][t])
                            if qb > 0:
                                oh = ohp.tile([128, QMAX * 128], bf16, tag="oh")
                                nc.vector.tensor_tensor(
                                    out=oh[:, 0 : qb * 128].rearrange(
                                        "p (q i) -> p q i", i=128),
                                    in0=dls[:, g0 + nid : g0 + qn].unsqueeze(2)
                                        .to_broadcast([128, qb, 128]),
                                    in1=iotabs[:].unsqueeze(1)
                                        .to_broadcast([128, qb, 128]),
                                    op=OP.is_equal,
                                )
                            if t not in pms:
                                pms[t] = psm.tile(
                                    [128, F], f32,
                                    name=f"pm{t - t0}", tag=f"pm{t - t0}")
                            pm = pms[t]
                            for k in range(qn):
                                q = g0 + k
                                ci, cj = cmap[q]
                                lhsT = (identbs[:] if k < nid else
                                        oh[:, (k - nid) * 128 : (k - nid + 1) * 128])
                                nc.tensor.matmul(
                                    out=pm[:],
                                    lhsT=lhsT,
                                    rhs=gt[ci][:, cj, 0:128],
                                    start=(w == wfirst[t] and k == 0),
                                    stop=(w == wlast[t] and k == qn - 1),
                                    skip_group_check=True,
                                )
                    for t in range(t0, t1):
                        if t in pms:
                            nc.vector.tensor_add(
                                out=accs[t][:], in0=accs[t][:], in1=pms[t][:]
                            )
                    out_cb(b, t0, t1)

            # ---- conv1 per-block tail: relu, transpose, conv2 linear, AG
            def cb1(b, t0, t1):
                for t in range(t0, t1):
                    hs = hsp.tile([128, F], f32, tag="hs")
                    nc.scalar.activation(
                        out=hs[:], in_=accs[t][:], func=AF.Copy,
                        scale=dinvsl[:, t : t + 1],
                    )
                    ptr = pst.tile([128, 128], f32, tag="ptr")
                    nc.tensor.transpose(out=ptr[:], in_=hs[:], identity=idents[:])
                    o1 = o1p.tile([128, 128], bf16, tag=f"o1T{t}")
                    nc.scalar.activation(
                        out=o1[:], in_=ptr[:], func=AF.Relu, bias=b1s[:, 0:1]
                    )
                    # conv2 linear on this tile
                    ph2 = psl.tile([128, F], f32, tag="ph")
                    nc.tensor.matmul(
                        out=ph2[:], lhsT=o1[:], rhs=W2s[:], start=True, stop=True
                    )
                    a2 = accp.tile([128, F], f32, tag=f"acc{t}")
                    nc.scalar.activation(
                        out=a2[:], in_=ph2[:], func=AF.Copy,
                        scale=dinvsl[:, t : t + 1],
                    )
                    accs[t] = a2
                    c2 = c2p.tile([128, F], bf16, tag="c2")
                    nc.vector.tensor_copy(out=c2[:], in_=a2[:])
                    k = int(np.searchsorted(cstart, t, side="right")) - 1
                    rt = t - int(cstart[k])
                    nc.sync.dma_start(
                        out=aginC[k][rt * 128 : (rt + 1) * 128, :], in_=c2[:]
                    )
                if stop_after is None or stop_after == "C":
                    # fire each AllGather chunk once its tiles are all written
                    for k in range(NCH):
                        lastt = int(cstart[k + 1]) - 1
                        if t0 <= lastt < t1:
                            nc.gpsimd.collective_compute(
                                "AllGather", OP.bypass,
                                replica_groups=[list(range(NCORES))],
                                ins=[aginC[k].ap().opt()],
                                outs=[tab2C[k][0 : NCORES * tch[k] * 128, :].opt()],
                            )

            # ---- conv2 per-tile tail: relu + pooling accumulate
            gps = psg.tile([128, F], f32, tag="gps")

            def cb2(t, first, last):
                o2 = o2p.tile([128, F], f32, tag="o2")
                nc.scalar.activation(
                    out=o2[:], in_=accs[t][:], func=AF.Copy,
                    scale=dinvsl[:, t : t + 1],
                )
                nc.vector.tensor_add(out=o2[:], in0=o2[:], in1=b2bs[:])
                nc.scalar.activation(out=o2[:], in_=o2[:], func=AF.Relu)
                ohf = ofp.tile([128, 128], f32, tag="ohf")
                nc.vector.tensor_tensor(
                    out=ohf[:],
                    in0=glocs[:, t : t + 1].to_broadcast([128, 128]),
                    in1=iotafs[:], op=OP.is_equal,
                )
                nc.tensor.matmul(
                    out=gps[:], lhsT=ohf[:], rhs=o2[:],
                    start=first, stop=last,
                    skip_group_check=True,
                )

            def dump_rows(dram_rows, od):
                tt = hsp.tile([128, F], bf16, tag="dbgb")
                nc.sync.dma_start(out=tt[:], in_=dram_rows)
                dd = hsp.tile([128, LAT], f32, tag="ms")
                nc.vector.tensor_copy(out=dd[:], in_=tt[:, 0:LAT])
                nc.sync.dma_start(out=od[:, :], in_=dd[:])

            def dump_sbuf(st, od):
                dd = hsp.tile([128, LAT], f32, tag="ms")
                nc.vector.tensor_copy(out=dd[:], in_=st[:, 0:LAT])
                nc.sync.dma_start(out=od[:, :], in_=dd[:])

            if stop_after == "A":
                dump_rows(tab1w[0][0:128, :], mu_d)
                dump_rows(tab1w[NW - 1][(WT - 1) * 128 : WT * 128, :], lv_d)
            if stop_after is None or stop_after in ("B", "C"):
                conv_mp(tab1w, s1["Qid"], s1["Qoh"], s1["offs"], s1["calls"],
                        cmap1, dl1s, idx1_d, cb1)
            if stop_after == "B":
                dump_sbuf(o1p.tile([128, 128], bf16, tag="o1T0"), mu_d)
                dump_sbuf(o1p.tile([128, 128], bf16, tag=f"o1T{NT - 1}"), lv_d)
            if stop_after == "C":
                dump_rows(tab2C[0][0:128, :], mu_d)
                dump_rows(tab2C[NCH - 1][127 * 128 : 128 * 128, :], lv_d)

            if stop_after is None:
                # ---- conv2 mp: window-major, per-(window,tile) PSUM chains
                Qid2, Qoh2, offs2 = s2["Qid"], s2["Qoh"], s2["offs"]
                wlast2 = {}
                for t in range(NT):
                    ws = [w for w in range(NCH) if Qid2[w][t] + Qoh2[w][t] > 0]
                    wlast2[t] = ws[-1] if ws else -1
                ndone = 0
                for t in range(NT):
                    if wlast2[t] < 0:  # no messages at all: pool the init acc
                        cb2(t, ndone == 0, ndone == NT - 1)
                        ndone += 1
                qsel2 = 0
                g = 0
                for w in range(NCH):
                    gt = {}
                    for (bb, ww, c0, c1) in s2["calls"]:
                        if ww != w:
                            continue
                        nb_ = c1 - c0
                        it = itp.tile([128, GC * 8], i16, tag="it")
                        nc.scalar.dma_start(
                            out=it[:, 0 : nb_ * 8],
                            in_=idx2_d[:, c0 * 8 : c1 * 8],
                        )
                        gb = gbp.tile([128, GC, 128], bf16, tag="gb")
                        nc.gpsimd.dma_gather(
                            gb[:, 0:nb_, :],
                            tab2C[w][:, :],
                            it[:, 0 : nb_ * 8],
                            nb_ * 128, nb_ * 128, 128,
                            single_packet=False,
                            queue_num=qsel2,
                        )
                        qsel2 ^= 1
                        gt[c0] = gb
                    for t in range(NT):
                        nid = int(Qid2[w][t])
                        qb = int(Qoh2[w][t])
                        qn = nid + qb
                        if qn == 0:
                            continue
                        g0 = int(offs2[w][t])
                        if qb > 0:
                            oh = ohp.tile([128, QMAX * 128], bf16, tag="oh")
                            nc.vector.tensor_tensor(
                                out=oh[:, 0 : qb * 128].rearrange(
                                    "p (q i) -> p q i", i=128),
                                in0=dl2s[:, g0 + nid : g0 + qn].unsqueeze(2)
                                    .to_broadcast([128, qb, 128]),
                                in1=iotabs[:].unsqueeze(1)
                                    .to_broadcast([128, qb, 128]),
                                op=OP.is_equal,
                            )
                        j = g % 4
                        g += 1
                        pm = psm.tile([128, F], f32,
                                      name=f"pm{j}", tag=f"pm{j}")
                        for k in range(qn):
                            ci, cj = cmap2[g0 + k]
                            lhsT = (identbs[:] if k < nid else
                                    oh[:, (k - nid) * 128 : (k - nid + 1) * 128])
                            nc.tensor.matmul(
                                out=pm[:],
                                lhsT=lhsT,
                                rhs=gt[ci][:, cj, 0:128],
                                start=(k == 0), stop=(k == qn - 1),
                                skip_group_check=True,
                            )
                        nc.vector.tensor_add(
                            out=accs[t][:], in0=accs[t][:], in1=pm[:]
                        )
                        if w == wlast2[t]:
                            cb2(t, ndone == 0, ndone == NT - 1)
                            ndone += 1

                # ---- mean + heads
                gm = hsp.tile([128, F], f32, tag="gm")
                nc.scalar.activation(
                    out=gm[:], in_=gps[:], func=AF.Copy, scale=icnts[:, 0:1]
                )
                pgt = pst.tile([128, 128], f32, tag="ptr")
                nc.tensor.transpose(out=pgt[:], in_=gm[:], identity=idents[:])
                gT = hsp.tile([128, 128], f32, tag="gT")
                nc.vector.tensor_copy(out=gT[:], in_=pgt[:])
                for Wd, bb, od in ((Wmus, bmubs, mu_d), (Wlvs, blvbs, lv_d)):
                    pmu = psl.tile([128, LAT], f32, tag="ph")
                    nc.tensor.matmul(
                        out=pmu[:], lhsT=gT[:], rhs=Wd[:], start=True, stop=True
                    )
                    ms = hsp.tile([128, LAT], f32, tag="ms")
                    nc.vector.tensor_add(out=ms[:], in0=pmu[:], in1=bb[:])
                    nc.sync.dma_start(out=od[:, :], in_=ms[:])

    nc.compile()
    return nc


def make_in_maps(pr, W1, b1, W2, b2, Wmu, bmu, Wlv, blv):
    F = pr["F"]
    iota = np.tile(np.arange(128, dtype=np.float32), (128, 1))
    shared = {
        "W1b": np.asarray(W1, np.float32).astype(BF16),
        "W2b": np.asarray(W2, np.float32).astype(BF16),
        "Wmu": np.asarray(Wmu, np.float32), "Wlv": np.asarray(Wlv, np.float32),
        "b1c": np.asarray(b1, np.float32).reshape(F, 1),
        "b2b": np.tile(np.asarray(b2, np.float32), (128, 1)),
        "bmub": np.tile(np.asarray(bmu, np.float32), (128, 1)),
        "blvb": np.tile(np.asarray(blv, np.float32), (128, 1)),
        "iotab": iota.astype(BF16), "iotaf": iota,
        "identf": np.eye(128, dtype=np.float32),
        "identb": np.eye(128, dtype=np.float32).astype(BF16),
    }
    maps = []
    for c in range(NCORES):
        m = dict(shared)
        m["xT"] = pr["xTs"][c]
        m["degw"] = pr["degws"][c]
        m["idx1"] = pr["s1"]["idx16"][c]
        m["dl1"] = pr["s1"]["dl"][c]
        m["idx2"] = pr["s2"]["idx16"][c]
        m["dl2"] = pr["s2"]["dl"][c]
        m["gloc"] = pr["glocw"][c]
        m["icnt"] = pr["icnt"][c]
        maps.append(m)
    return maps


def kernel(x, edge_index, batch, W1, b1, W2, b2, Wmu, bmu, Wlv, blv):
    n_graphs = 1024
    pr = prep_host(x, edge_index, batch, n_graphs)
    LAT = np.asarray(Wmu).shape[1]
    nc = build_nc(pr, LAT)
    in_maps = make_in_maps(pr, W1, b1, W2, b2, Wmu, bmu, Wlv, blv)
    res = run_bass_kernel_spmd(nc, in_maps, core_ids=list(range(NCORES)))
    mu = np.concatenate([res.results[c]["mu_o"] for c in range(NCORES)], axis=0)
    lv = np.concatenate([res.results[c]["lv_o"] for c in range(NCORES)], axis=0)
    return (mu.astype(np.float32), lv.astype(np.float32))
